# revision 35
# baseline (speedup 1.0000x reference)
"""CVRP decoder kernel for 8x TRN2 NeuronCores.

Full computation on device, data-parallel over batch (2 batches/core):
  - lazy-mask MLP (B*P*N rows of 3->64->64->1) as fp16 matmuls with p-pair
    packing (2 pomo rows per 128-partition matmul, 3 pairs per SBUF tile at
    32-aligned bases; PE quadrant 3 is unusable)
  - masked multi-head attention + FiLM'd queries (fp16 matmuls, f32 softmax)
  - pointer scores + softmax (fp16 matmul, ACT tanh/exp)
Host does only the tiny query-side ops (P=100 rows: FiLM MLP, q proj, lazy
query features) and fp16 layout prep.  The axon wire is ~40 MB/s, so
everything shipped is fp16 (10-bit mantissa ~ f32r accuracy) and the all-zero
ninf_mask is elided entirely (a with-ninf fallback program is compiled
lazily if a nonzero mask ever shows up).  Output travels as bf16 (fp16
subnormals would distort tiny softmax tails).
"""
import os
import numpy as np
import ml_dtypes

import jax
from jax.experimental.shard_map import shard_map
from jax.sharding import Mesh, PartitionSpec

import concourse.bass as bass
import concourse.tile as tile
from concourse import bacc, bass2jax, mybir
from concourse.masks import make_identity

E, H, D = 128, 8, 16
QDIM, DYN, HID = 16, 3, 64
ALPHA = 1.0
CLIP = 10.0
NCORES = 8
B, P, N = 16, 100, 1000
BPC = B // NCORES          # batches per core
NPAIR = P // 2             # 50 p-pairs
NT = 2                     # n tiles of 500
NW = N // NT
NC8 = 8                    # n chunks of 125 for attn@v
NCW = N // NC8

f32 = mybir.dt.float32
f16 = mybir.dt.float16
f32r = mybir.dt.float32r
bf16 = mybir.dt.bfloat16
AF = mybir.ActivationFunctionType
OP = mybir.AluOpType

nbf = ml_dtypes.bfloat16


def _build_nc(with_ninf):
    nc = bacc.Bacc("TRN2", target_bir_lowering=False, debug=False,
                   num_devices=NCORES)
    nodesT = nc.dram_tensor("nodesT", [BPC, E, N], f16, kind="ExternalInput").ap()
    dyn6 = nc.dram_tensor("dyn6", [BPC, 6 * NPAIR, N], f16, kind="ExternalInput").ap()
    qT = nc.dram_tensor("qT", [BPC, 128, P], f16, kind="ExternalInput").ap()
    c1T = nc.dram_tensor("c1T", [BPC, 128, NPAIR], f32, kind="ExternalInput").ap()
    if with_ninf:
        ninf = nc.dram_tensor("ninf", [BPC, P, N], f32, kind="ExternalInput").ap()
    # packed weight blobs: each dma_start costs ~650ns of issue time, so
    # ship few large tensors. wf16 cols: w1rep 0:128 | w2d 128:256 |
    # wk 256:640 | qsel 640:1024 | wv 1024:1152 | wcomb 1152:1280 | w3c 1280
    wf16 = nc.dram_tensor("wf16", [128, 1281], f16, kind="ExternalInput").ap()
    wf32 = nc.dram_tensor("wf32", [128, 3], f32, kind="ExternalInput").ap()
    idpack = nc.dram_tensor("idpack", [P, 2 * P], f32r, kind="ExternalInput").ap()
    probs = nc.dram_tensor("probs", [BPC, P, N], bf16, kind="ExternalOutput").ap()

    NDT = (NPAIR + 2) // 3  # 17 dyn tiles per batch (3 pairs @ bases 0/32/64)

    with tile.TileContext(nc) as tc:
        with (
            tc.tile_pool(name="cw", bufs=1) as cw,
            tc.tile_pool(name="io", bufs=2) as io,
            tc.tile_pool(name="dynp", bufs=2 * NDT) as dynp,
            tc.tile_pool(name="kvp", bufs=2) as kvp,
            tc.tile_pool(name="vp", bufs=2 * NC8) as vp,
            tc.tile_pool(name="wrk", bufs=3) as wrk,
            tc.tile_pool(name="msk", bufs=2 * NT) as msk,
            tc.tile_pool(name="st", bufs=8) as st,
            tc.tile_pool(name="psA", bufs=3, space="PSUM") as psA,
            tc.tile_pool(name="psB", bufs=2, space="PSUM") as psB,
        ):
            # ---- constants (packed blobs, 3 DMAs)
            wf16_sb = cw.tile([128, 1281], f16, tag="wf16")
            nc.sync.dma_start(wf16_sb[:], wf16)
            wf32_sb = cw.tile([128, 3], f32, tag="wf32")
            nc.sync.dma_start(wf32_sb[:], wf32)
            idpk_sb = cw.tile([P, 2 * P], f32r, tag="idpk")
            nc.sync.dma_start(idpk_sb[:], idpack)
            w1rep_sb = wf16_sb[:, 0:128]
            w2d_sb = wf16_sb[:, 128:256]
            wk_sb = [wf16_sb[:, 256 + 128 * s:384 + 128 * s] for s in range(3)]
            qsel_sb = [wf16_sb[:, 640 + 128 * s:768 + 128 * s] for s in range(3)]
            wv_sb = wf16_sb[:, 1024:1152]
            wcomb_sb = wf16_sb[:, 1152:1280]
            w3c_sb = wf16_sb[:, 1280:1281]
            b2d_sb = wf32_sb[:, 0:1]
            b3bc_sb = wf32_sb[:, 1:2]
            bcomb_sb = wf32_sb[:, 2:3]
            idm1 = idpk_sb[:, 0:P]
            idp1 = idpk_sb[:, P:2 * P]
            wcomb_r = cw.tile([128, 128], f32r, tag="wcombr")
            nc.vector.tensor_copy(wcomb_r[:], wcomb_sb)
            ident = cw.tile([P, P], f32, tag="ident")
            make_identity(nc, ident[:])
            ones16 = cw.tile([P, D], f32, tag="ones16")
            nc.gpsimd.memset(ones16[:], 1.0)
            # L3 stationary bank: one [128,128] block per pair, W3 in
            # columns (2i, 2i+1)
            w3b_sb = cw.tile([128, NPAIR * 128], f16, tag="w3b")
            nc.gpsimd.memset(w3b_sb[:], 0.0)
            for i in range(NPAIR):
                c = 128 * i + 2 * i
                nc.gpsimd.tensor_copy(w3b_sb[0:HID, c:c + 1], w3c_sb[0:HID, :])
                nc.gpsimd.tensor_copy(w3b_sb[HID:128, c + 1:c + 2],
                                      w3c_sb[HID:128, :])

            # ---- per-batch input tiles
            nt_sb, ninf_sb, qt_sb, c1_sb = {}, {}, {}, {}
            dyn_sb = {}
            for b in range(BPC):
                # small tensors first so they don't queue behind the dyn bulk
                nt_sb[b] = io.tile([E, N], f16, tag="nodes", name="nt")
                nc.sync.dma_start(nt_sb[b][:], nodesT[b])
                if with_ninf:
                    ninf_sb[b] = io.tile([P, N], f32, tag="ninf", name="ninfs")
                    nc.sync.dma_start(ninf_sb[b][:], ninf[b])
                qtile = io.tile([128, P], f16, tag="qt", name="qts")
                nc.sync.dma_start(qtile[:], qT[b])
                c1_sb[b] = io.tile([128, NPAIR], f32, tag="c1", name="c1s")
                nc.sync.dma_start(c1_sb[b][:], c1T[b])
                # expand q into 3 head-slot tiles via PE selection matmuls
                qt_sb[b] = []
                for s in range(3):
                    ps_q = psA.tile([128, P], f32, tag="mm2")
                    nc.tensor.matmul(ps_q[:], qsel_sb[s], qtile[:],
                                     start=True, stop=True)
                    q3 = wrk.tile([128, P], f32r, tag=f"q3_{s}", name="q3t",
                                  bufs=2)
                    nc.vector.tensor_copy(q3[:], ps_q[:])
                    qt_sb[b].append(q3)
                dtiles = []
                for t in range(NDT):
                    dtile = dynp.tile([128, N], f16, tag="dyn", name="dt",
                                      bufs=2 * NDT)
                    for a in range(3):
                        i = 3 * t + a
                        if i >= NPAIR:
                            break
                        q_eng = nc.sync if i % 2 == 0 else nc.scalar
                        q_eng.dma_start(dtile[32 * a:32 * a + 6, :],
                                        dyn6[b, 6 * i:6 * i + 6, :])
                    dtiles.append(dtile)
                dyn_sb[b] = dtiles

            mask_sb = {}
            kt_sb, v_sb, ntr_sb = {}, {}, {}

            # greedy relu router: direct on ACT/DVE, or staged copy + relu on
            # the otherwise-idle GpSimd. Costs are cost-model estimates (ns).
            est = {"ACT": 35000.0, "DVE": 50000.0, "POOL": 21000.0}

            def emit_relu(h, ps, bias_ap):
                a_c = est["ACT"] + 600
                d_c = est["DVE"] + 642
                if a_c <= d_c:
                    nc.scalar.activation(h[:], ps[:], AF.Relu, bias=bias_ap)
                    est["ACT"] += 600
                else:
                    nc.vector.tensor_scalar(h[:], ps[:], bias_ap, 0.0,
                                            OP.add, OP.max)
                    est["DVE"] += 642

            def mlp_phase(b):
                ps_m_l = {}
                for j in range(NT):
                    ps_m = psB.tile([128, NW], f32, tag="acc")
                    ps1_q, ps2_q = {}, {}
                    # 3-stage software pipeline: L1(i) | relu+L2(i-1) |
                    # relu+L3(i-2) so PE never queue-blocks on a relu
                    for step in range(NPAIR + 2):
                        i0, i1, i2 = step, step - 1, step - 2
                        if i0 < NPAIR:
                            t, a = i0 // 3, i0 % 3
                            rhs = dyn_sb[b][t][32 * a:32 * a + 6,
                                               j * NW:(j + 1) * NW]
                            ps1 = psA.tile([128, NW], f32, tag="mm1")
                            nc.tensor.matmul(
                                ps1[:], w1rep_sb[32 * a:32 * a + 6, :],
                                rhs, start=True, stop=True)
                            ps1_q[i0] = ps1
                        if 0 <= i1 < NPAIR:
                            h1 = wrk.tile([128, NW], f16, tag="h1", bufs=4)
                            p1 = ps1_q.pop(i1)
                            emit_relu(h1, p1, c1_sb[b][:, i1:i1 + 1])
                            ps2 = psA.tile([128, NW], f32, tag="mm2")
                            nc.tensor.matmul(ps2[:], w2d_sb, h1[:],
                                             start=True, stop=True)
                            ps2_q[i1] = ps2
                        if 0 <= i2:
                            h2 = wrk.tile([128, NW], f16, tag="h2", bufs=4)
                            p2 = ps2_q.pop(i2)
                            emit_relu(h2, p2, b2d_sb)
                            nc.tensor.matmul(ps_m[:],
                                             w3b_sb[:, 128 * i2:128 * i2 + 128],
                                             h2[:],
                                             start=(i2 == 0),
                                             stop=(i2 == NPAIR - 1))
                    ps_m_l[j] = ps_m
                # softplus(z + b3) composed as Ln(Exp(z + b3) + 1); grouped
                # after both j-tiles to avoid ACT func-set thrash mid-stream
                for j in range(NT):
                    ez = wrk.tile([P, NW], f32, tag="ez")
                    nc.scalar.activation(ez[:], ps_m_l[j][0:P, :], AF.Exp,
                                         bias=b3bc_sb[0:P, :])
                    sp = msk.tile([P, NW], f32r, tag="mask", name="sp")
                    nc.scalar.activation(sp[:], ez[:], AF.Ln, bias=1.0)
                    if with_ninf:
                        m = msk.tile([P, NW], f32r, tag="nmask", name="nm")
                        nc.vector.tensor_sub(
                            m[:], ninf_sb[b][:, j * NW:(j + 1) * NW], sp[:])
                        mask_sb[(b, j)] = ("add", m)
                    else:
                        mask_sb[(b, j)] = ("sub", sp)

            def kv_phase(b):
                kt_sb[b] = []
                for s in range(3):
                    ktile = kvp.tile([128, N], f32r, tag=f"kt{s}", name="kts")
                    kt_sb[b].append(ktile)
                for j in range(NT):
                    for s in range(3):
                        ps_k = psA.tile([128, NW], f32, tag="mm1")
                        nc.tensor.matmul(ps_k[:], wk_sb[s],
                                         nt_sb[b][:, j * NW:(j + 1) * NW],
                                         start=True, stop=True)
                        nc.vector.tensor_copy(
                            kt_sb[b][s][:, j * NW:(j + 1) * NW], ps_k[:])
                ntr_sb[b] = kvp.tile([128, N], f32r, tag="ntr", name="ntr")
                nc.vector.tensor_copy(ntr_sb[b][:], nt_sb[b][:])
                v_sb[b] = []
                for c in range(NC8):
                    ps_v = psA.tile([NCW, 128], f32, tag="mm2")
                    nc.tensor.matmul(
                        ps_v[:],
                        nt_sb[b][:, c * NCW:(c + 1) * NCW],
                        wv_sb, start=True, stop=True)
                    vt = vp.tile([NCW, 128], f32r, tag="v")
                    nc.vector.tensor_copy(vt[:], ps_v[:])
                    v_sb[b].append(vt)

            def att_phase(b):
                ps_oc = psB.tile([P, 128], f32, tag="acc")
                an_state = {}
                rec_l = {}
                # pipeline by one head: scores/softmax(hh) overlap
                # transpose+attn@v(hh-1)
                for hh in range(H + 1):
                    if hh < H:
                        s, g = hh % 3, hh // 3
                        qt = qt_sb[b][s]
                        kt = kt_sb[b][s]
                        e_t = []
                        rs = []
                        for j in range(NT):
                            ps_s = psA.tile([P, NW], f32, tag="mm1")
                            nc.tensor.matmul(
                                ps_s[:],
                                qt[32 * g:32 * g + D, :],
                                kt[32 * g:32 * g + D, j * NW:(j + 1) * NW],
                                start=True, stop=False)
                            op, mt = mask_sb[(b, j)]
                            nc.tensor.matmul(
                                ps_s[:], idp1 if op == "add" else idm1,
                                mt[:], start=False, stop=True)
                            ev = wrk.tile([P, NW], f32, tag="ev", bufs=5)
                            r = st.tile([P, 1], f32, tag="rs")
                            nc.scalar.activation(ev[:], ps_s[:], AF.Exp,
                                                 accum_out=r[:])
                            e_t.append(ev)
                            rs.append(r)
                        tot = st.tile([P, 1], f32, tag="tot")
                        nc.vector.tensor_add(tot[:], rs[0][:], rs[1][:])
                        rec = st.tile([P, 1], f32, tag="rec", bufs=10)
                        nc.vector.reciprocal(rec[:], tot[:])
                        rec_l[hh] = rec
                        an_state[hh] = e_t
                    if hh >= 1:
                        h = hh - 1
                        an_t = an_state.pop(h)
                        at_q = {}
                        for c in range(NC8):
                            jj = c // (NC8 // NT)
                            off = (c % (NC8 // NT)) * NCW
                            ps_t = psA.tile([NCW, P], f32, tag="mm2")
                            nc.tensor.transpose(ps_t[:],
                                                an_t[jj][:, off:off + NCW],
                                                ident[:])
                            at = wrk.tile([NCW, P], f32r, tag="at", bufs=4)
                            nc.vector.tensor_copy(at[:], ps_t[:])
                            at_q[c] = at
                            if c >= 1:
                                nc.tensor.matmul(
                                    ps_oc[:, D * h:D * h + D],
                                    at_q.pop(c - 1)[:],
                                    v_sb[b][c - 1][:, D * h:D * h + D],
                                    start=(h == 0 and c - 1 == 0), stop=False)
                        nc.tensor.matmul(
                            ps_oc[:, D * h:D * h + D],
                            at_q.pop(NC8 - 1)[:],
                            v_sb[b][NC8 - 1][:, D * h:D * h + D],
                            start=False,
                            stop=(h == H - 1))
                # normalization deferred: oc = ps_oc * R where
                # R[p, 16h+d] = 1/rowsum_h[p]
                Rt = wrk.tile([P, 128], f32, tag="Rt", bufs=2)
                for h in range(H):
                    nc.gpsimd.tensor_scalar(Rt[:, D * h:D * h + D],
                                            ones16[:], rec_l[h][:], None,
                                            OP.mult)
                oc = wrk.tile([P, 128], f32, tag="oc")
                nc.vector.tensor_tensor(oc[:], ps_oc[:], Rt[:], OP.mult)
                ps_t2 = psA.tile([128, P], f32, tag="mm2")
                nc.tensor.transpose(ps_t2[:], oc[:], ident[:])
                ocT = wrk.tile([128, P], f32r, tag="ocT")
                nc.vector.tensor_copy(ocT[:], ps_t2[:])
                ps_mh = psB.tile([128, P], f32, tag="acc")
                nc.tensor.matmul(ps_mh[:], wcomb_r[:],
                                 ocT[:], start=True, stop=True)
                mhT = wrk.tile([128, P], f32r, tag="mhT")
                nc.vector.tensor_scalar(mhT[:], ps_mh[:], bcomb_sb, None,
                                        OP.add)
                return mhT

            def ptr_phase(b, mhT):
                pe_t, prs = [], []
                for j in range(NT):
                    ps_p = psA.tile([P, NW], f32, tag="mm1")
                    nc.tensor.matmul(
                        ps_p[:], mhT[:],
                        ntr_sb[b][:, j * NW:(j + 1) * NW],
                        start=True, stop=True)
                    pt = wrk.tile([P, NW], f32, tag="pt")
                    nc.scalar.activation(pt[:], ps_p[:], AF.Tanh,
                                         scale=float(1.0 / np.sqrt(E)))
                    pev = wrk.tile([P, NW], f32, tag="pe")
                    r = st.tile([P, 1], f32, tag="prs")
                    if with_ninf:
                        pa = wrk.tile([P, NW], f32, tag="pa")
                        nc.vector.scalar_tensor_tensor(
                            pa[:], pt[:], CLIP,
                            ninf_sb[b][:, j * NW:(j + 1) * NW],
                            OP.mult, OP.add)
                        nc.scalar.activation(pev[:], pa[:], AF.Exp,
                                             accum_out=r[:])
                    else:
                        nc.scalar.activation(pev[:], pt[:], AF.Exp, scale=CLIP,
                                             accum_out=r[:])
                    pe_t.append(pev)
                    prs.append(r)
                ptot = st.tile([P, 1], f32, tag="ptot")
                nc.vector.tensor_add(ptot[:], prs[0][:], prs[1][:])
                prec = st.tile([P, 1], f32, tag="prec")
                nc.vector.reciprocal(prec[:], ptot[:])
                for j in range(NT):
                    po = wrk.tile([P, NW], bf16, tag="po")
                    nc.gpsimd.tensor_scalar(po[:], pe_t[j][:], prec[:], None,
                                            OP.mult)
                    nc.sync.dma_start(probs[b, :, j * NW:(j + 1) * NW], po[:])

            for b in range(BPC):
                mlp_phase(b)
                kv_phase(b)
            mh_all = [att_phase(b) for b in range(BPC)]
            for b in range(BPC):
                ptr_phase(b, mh_all[b])
    nc.compile()
    return nc


# --------------------------------------------------------------------------
# cached PJRT runner (replicates bass2jax.run_bass_via_pjrt's multi-core path
# but builds the jitted executable once per program instead of per call)

class _Runner:
    def __init__(self, nc):
        bass2jax.install_neuronx_cc_hook()
        self.nc = nc
        in_names, out_names, out_avals, zero_shapes = [], [], [], []
        partition_name = (nc.partition_id_tensor.name
                          if nc.partition_id_tensor else None)
        for alloc in nc.m.functions[0].allocations:
            if not isinstance(alloc, mybir.MemoryLocationSet):
                continue
            name = alloc.memorylocations[0].name
            if alloc.kind == "ExternalInput":
                if name != partition_name:
                    in_names.append(name)
            elif alloc.kind == "ExternalOutput":
                out_names.append(name)
                shape = tuple(alloc.tensor_shape)
                dtype = mybir.dt.np(alloc.dtype)
                out_avals.append(jax.core.ShapedArray(shape, dtype))
                zero_shapes.append((shape, dtype))
        self.n_params = len(in_names)
        self.in_names = list(in_names)
        self.out_names = out_names
        self.out_avals = out_avals
        self.zero_shapes = zero_shapes
        all_names = in_names + out_names
        if partition_name is not None:
            all_names.append(partition_name)

        def _body(*args):
            operands = list(args)
            if partition_name is not None:
                operands.append(bass2jax.partition_id_tensor())
            outs = bass2jax._bass_exec_p.bind(
                *operands,
                out_avals=tuple(out_avals),
                in_names=tuple(all_names),
                out_names=tuple(out_names),
                lowering_input_output_aliases=(),
                sim_require_finite=True,
                sim_require_nnan=True,
                nc=nc,
            )
            return tuple(outs)

        devices = jax.devices()[:NCORES]
        mesh = Mesh(np.asarray(devices), ("core",))
        n_out = len(out_names)
        donate = tuple(range(self.n_params, self.n_params + n_out))
        in_specs = (PartitionSpec("core"),) * (self.n_params + n_out)
        out_specs = (PartitionSpec("core"),) * n_out
        self.sharded = jax.jit(
            shard_map(_body, mesh=mesh, in_specs=in_specs,
                      out_specs=out_specs, check_rep=False),
            donate_argnums=donate, keep_unused=True)

    def __call__(self, in_maps):
        concat_in = [
            np.concatenate([np.asarray(m[name]) for m in in_maps], axis=0)
            for name in self.in_names]
        zeros = [np.zeros((NCORES * s[0], *s[1:]), d)
                 for s, d in self.zero_shapes]
        out_arrs = self.sharded(*concat_in, *zeros)
        return {name: np.asarray(out_arrs[i])
                for i, name in enumerate(self.out_names)}


_RUNNERS = {}
LAST_RES = None


def _get_runner(with_ninf):
    if with_ninf not in _RUNNERS:
        _RUNNERS[with_ninf] = _Runner(_build_nc(with_ninf))
    return _RUNNERS[with_ninf]


def _prep(inp, with_ninf):
    """Host-side prep: small query-side math + fp16 layout."""
    q_in = np.concatenate([inp["encoded_last_node"],
                           inp["load"][:, :, None]], axis=-1).astype(np.float32)
    q = (q_in @ inp["Wq_last"]).astype(np.float32)
    g = np.maximum(q_in @ inp["film_W1"] + inp["film_b1"], 0.0) @ inp["film_W2"] \
        + inp["film_b2"]
    gamma = (2.0 / (1.0 + np.exp(-g))).astype(np.float32)
    # fold the 1/sqrt(D) score scale into q
    q = q * gamma * np.float32(1.0 / np.sqrt(D))              # (B,P,H*D)
    qf = q_in @ inp["lazy_q_W"] + inp["lazy_q_b"]
    c1 = (qf @ inp["lm_W1"][DYN:] + inp["lm_b1"]).astype(np.float32)  # (B,P,HID)

    qT = np.ascontiguousarray(q.transpose(0, 2, 1)).astype(np.float16)
    c1T = np.ascontiguousarray(c1.reshape(B, NPAIR, 128).transpose(0, 2, 1))

    dyn_f16 = inp["dyn_features"].astype(np.float16)          # (B,P,N,3)
    dyn6 = np.ascontiguousarray(
        dyn_f16.transpose(0, 1, 3, 2)).reshape(B, 6 * NPAIR, N)
    nodesT = np.ascontiguousarray(
        inp["encoded_nodes"].transpose(0, 2, 1)).astype(np.float16)

    w1rep = np.zeros((128, 128), np.float16)
    for a in range(3):
        w1rep[32 * a:32 * a + DYN, 0:HID] = inp["lm_W1"][:DYN]
        w1rep[32 * a + DYN:32 * a + 2 * DYN, HID:128] = inp["lm_W1"][:DYN]
    w2d = np.zeros((128, 128), np.float16)
    w2d[0:HID, 0:HID] = inp["lm_W2"]
    w2d[HID:, HID:] = inp["lm_W2"]
    w3c = np.tile(inp["lm_W3"][:, 0], 2)[:, None].astype(np.float16)
    b2d = np.tile(inp["lm_b2"], 2)[:, None].astype(np.float32)
    b3bc = np.full((128, 1), inp["lm_b3"][0], np.float32)
    wk3 = [np.zeros((128, 128), np.float16) for _ in range(3)]
    qsel = [np.zeros((128, 128), np.float16) for _ in range(3)]
    for h in range(H):
        s, gi = h % 3, h // 3
        wk3[s][:, 32 * gi:32 * gi + D] = inp["Wk"][:, D * h:D * h + D]
        for d in range(D):
            qsel[s][D * h + d, 32 * gi + d] = 1.0
    wf16 = np.concatenate(
        [w1rep, w2d, wk3[0], wk3[1], wk3[2], qsel[0], qsel[1], qsel[2],
         inp["Wv"].astype(np.float16), inp["W_comb"].astype(np.float16),
         w3c], axis=1)
    wf32 = np.concatenate(
        [b2d, b3bc, inp["b_comb"][:, None].astype(np.float32)], axis=1)
    idpack = np.concatenate(
        [-np.eye(P, dtype=np.float32), np.eye(P, dtype=np.float32)], axis=1)
    shared = {"wf16": wf16, "wf32": wf32, "idpack": idpack}
    if with_ninf:
        ninf = np.ascontiguousarray(inp["ninf_mask"]).astype(np.float32)
    in_maps = []
    for c in range(NCORES):
        sl = slice(c * BPC, (c + 1) * BPC)
        m = {"nodesT": nodesT[sl], "dyn6": dyn6[sl], "qT": qT[sl],
             "c1T": c1T[sl]}
        if with_ninf:
            m["ninf"] = ninf[sl]
        m.update(shared)
        in_maps.append(m)
    return in_maps


def kernel(**inputs):
    global LAST_RES
    inp = {k: np.asarray(v) for k, v in inputs.items()}
    with_ninf = bool(np.any(inp["ninf_mask"]))
    in_maps = _prep(inp, with_ninf)
    if os.environ.get("KBENCH_TRACE", "0") != "0":
        from concourse.bass_utils import run_bass_kernel_spmd
        runner = _get_runner(with_ninf)   # reuse cached program build
        res = run_bass_kernel_spmd(runner.nc, in_maps, list(range(NCORES)),
                                   trace=True)
        LAST_RES = res
        probs = np.concatenate(
            [np.asarray(res.results[c]["probs"]) for c in range(NCORES)],
            axis=0)
        return probs.astype(np.float32)
    in_maps = in_maps
    runner = _get_runner(with_ninf)
    outs = runner(in_maps)
    LAST_RES = None
    probs = outs["probs"].reshape(B, P, N)
    return probs.astype(np.float32)
